# revision 1
# baseline (speedup 1.0000x reference)
"""ConvTranspose2d(64->64, k=3, s=1, p=0) on (2, 64, 1024, 1024) fp32.

out[b, o, p, q] = sum_{c,kh,kw} weight[c, o, kh, kw] * x[b, c, p-kh, q-kw]
out shape (2, 64, 1026, 1026).

Strategy (8 NeuronCores, pure data parallel over batch x H):
  - Each core handles one batch and a quarter of the output rows.
  - Output rows are processed in PAIRS (2j, 2j+1). Input rows are stacked
    in pairs U_j = [x[2j]; x[2j+1]] as SBUF tiles of 128 partitions
    (partition = 64*u + c, u = row-of-pair, c = channel).
  - Per output pair and kw-shift s, two K=128 matmuls accumulate in PSUM:
      A_s^T @ U_j     with A_s = [[W0s, W1s], [0, W0s]]
      B_s^T @ U_{j-1} with B_s = [[W2s, 0], [W1s, W2s]]
    where Wks = weight[:, :, k, s]. Output partition = 64*v + o (v =
    out-row-of-pair). 1026 output columns are split in 3 PSUM chunks of 342.
  - Matmuls run as float32r (TF32-class, full PE rate at N>=256), PSUM
    accumulates fp32. Column shifts use 2 zero pad columns on each side of
    the 1028-wide row tiles (pads baked into the host-packed input).
"""

import numpy as np

B = 2
C = 64
H = 1024
W = 1024
HO = 1026
WO = 1026
WP = W + 4  # 2 zero pad cols each side
NPAIR = 129  # output row pairs computed per core
NTILE = NPAIR + 1  # U tiles per core incl. leading halo tile
J0S = (0, 128, 256, 384)  # first output pair per core (within a batch)
VALID = (128, 128, 128, 129)  # pairs consumed from each core
CHUNKS = ((0, 342), (342, 342), (684, 342))

U_BUFS = 6
O_BUFS = 4
PS_BUFS = 2

_CACHE = {}


def _build(npair=NPAIR, reps=1, u_bufs=U_BUFS, o_bufs=O_BUFS, ps_bufs=PS_BUFS):
    import concourse.bacc as bacc
    import concourse.mybir as mybir
    from concourse.tile import TileContext

    F32 = mybir.dt.float32
    F32R = mybir.dt.float32r

    nc = bacc.Bacc()
    xs = nc.dram_tensor("xs", [npair + 1, 128, WP], F32R, kind="ExternalInput")
    ws = nc.dram_tensor("ws", [128, 768], F32R, kind="ExternalInput")
    outs = nc.dram_tensor("outs", [npair, 128, WO], F32, kind="ExternalOutput")
    with TileContext(nc) as tc:
        with (
            tc.tile_pool(name="w", bufs=1) as wp,
            tc.tile_pool(name="u", bufs=u_bufs) as up,
            tc.tile_pool(name="ob", bufs=o_bufs) as ob,
            tc.tile_pool(name="ps", bufs=ps_bufs, space="PSUM") as pp,
        ):
            wsb = wp.tile([128, 768], F32R)
            nc.sync.dma_start(out=wsb, in_=ws[:, :])
            for _ in range(reps):
                prev = None
                for j in range(npair):
                    if prev is None:
                        prev = up.tile([128, WP], F32R, tag="u")
                        nc.sync.dma_start(out=prev, in_=xs[0])
                    cur = up.tile([128, WP], F32R, tag="u")
                    nc.sync.dma_start(out=cur, in_=xs[j + 1])
                    osb = ob.tile([128, WO], F32, tag="ob")
                    for ci, (n0, nch) in enumerate(CHUNKS):
                        ps = pp.tile([128, nch], F32, tag=f"c{ci}")
                        k = 0
                        for g, ut in ((0, cur), (1, prev)):
                            for s in range(3):
                                i0 = (g * 3 + s) * 128
                                nc.tensor.matmul(
                                    ps[:, :],
                                    wsb[:, i0 : i0 + 128],
                                    ut[:, n0 + 2 - s : n0 + 2 - s + nch],
                                    start=(k == 0),
                                    stop=(k == 5),
                                )
                                k += 1
                        nc.vector.tensor_copy(out=osb[:, n0 : n0 + nch], in_=ps[:, :])
                    nc.sync.dma_start(out=outs[j], in_=osb)
                    prev = cur
    nc.compile()
    return nc


def _pack_weight(weight):
    """weight (64, 64, 3, 3) fp32 -> (128, 768) stationary blocks.

    ws[64*u + c, (3*g + s)*128 + 64*v + o] = weight[c, o, v - u + 2*g, s]
    when 0 <= v - u + 2*g <= 2 else 0.
    """
    wsb = np.zeros((128, 768), np.float32)
    for g in (0, 1):
        for s in range(3):
            col0 = (3 * g + s) * 128
            for u in (0, 1):
                for v in (0, 1):
                    kh = v - u + 2 * g
                    if 0 <= kh <= 2:
                        wsb[64 * u : 64 * u + 64, col0 + 64 * v : col0 + 64 * v + 64] = (
                            weight[:, :, kh, s]
                        )
    return wsb


def _pack_core_input(xb, j0):
    """xb (64, 1024, 1024) fp32 -> xs (130, 128, 1028) for pairs j0..j0+128.

    xs[t] holds U_{j0+t-1}: rows 2*(j0+t-1) and +1, zero outside [0, H),
    with 2 zero pad columns on both sides.
    """
    xs = np.zeros((NTILE * 2, 64, WP), np.float32)
    r0 = 2 * j0 - 2  # first source row
    lo = max(0, r0)
    hi = min(H, r0 + 2 * NTILE)
    xs[lo - r0 : hi - r0, :, 2 : 2 + W] = xb[:, lo:hi, :].transpose(1, 0, 2)
    return xs.reshape(NTILE, 128, WP)


def kernel(x, weight):
    from concourse.bass_utils import run_bass_kernel_spmd

    x = np.ascontiguousarray(x, dtype=np.float32)
    weight = np.ascontiguousarray(weight, dtype=np.float32)

    if "nc" not in _CACHE:
        _CACHE["nc"] = _build()
    nc = _CACHE["nc"]

    wsb = _pack_weight(weight)
    in_maps = []
    for core in range(8):
        b, k = divmod(core, 4)
        in_maps.append({"xs": _pack_core_input(x[b], J0S[k]), "ws": wsb})

    res = run_bass_kernel_spmd(nc, in_maps, core_ids=list(range(8)))

    out = np.empty((B, C, HO, WO), np.float32)
    for core in range(8):
        b, k = divmod(core, 4)
        nv = VALID[k]
        rows = res.results[core]["outs"].reshape(NPAIR * 2, C, WO)
        out[b, :, 2 * J0S[k] : 2 * (J0S[k] + nv), :] = rows[: 2 * nv].transpose(1, 0, 2)
    return out
